# revision 1
# baseline (speedup 1.0000x reference)
"""Trainium2 Bass kernel: soft-top-k-masked pseudo-diagonal fully connected layer.

Computes, for x [16, 1024, 768], V [2304, 768], alpha [2304]:
    m  = dykstra_capped_simplex(alpha / 0.01, k=231, 50 iters)        # [2304]
    W[o, j] = m[(o - j) % 2304] * V[(o - j) % 2304, j]                # [2304, 768]
    out = x @ W.T                                                     # [16, 1024, 2304]

Key identities used:
  * Dykstra on the capped simplex reduces to a scalar recursion on w:
        w_1     = z + (k - sum(z)) / n
        w_{i+1} = w_i + (k - sum(clip(w_i, 0, 1))) / n     (49 times)
        m       = clip(w_50, 0, 1)
    (p is always a constant vector and y+q == w, so only w and the scalar
    sum survive.)  clip+sum fuse into one DVE op (scalar_tensor_tensor with
    accum_out); the cross-partition sum broadcast is a ones-matmul on the PE.
  * The scatter .at[rows, cols].add is a bijection per column, so
        W.T[j, o] = (m * V).T[j, (o - j) mod 2304]
    i.e. row j of W.T is row j of V.T cyclically shifted right by j, scaled
    by a shifted copy of m.  V.T is stored wrap-extended in DRAM as
    ext[768, 2432] with ext[j, 128 + c] = V.T[j, c mod 2304]; a W.T j-tile
    is then two DMAs with partition stride 2431 (skewed access pattern).
    This raw roll is *independent of m*, so it runs concurrently with the
    Dykstra iteration; m is applied afterwards as
        W.T_tile[b] *= m_skew[b],   m_skew[b][dj, o] = m[(o-128b-dj) % 2304]
    where m_skew is a skewed broadcast of m loaded from a 130x-replicated
    m_rep buffer with partition stride 2303 (== -1 mod 2304).

Sharding: data-parallel over the 16384 tokens -> 2048 tokens per core on 8
cores; V/alpha replicated (per the sharding hint). The x/V/W datapath is
float32r (fp22 on the PE; 1 cycle/row for moving dims >= 256, 1.5
cycles/row transposes); the Dykstra recursion stays exact float32.
"""

import numpy as np

from concourse import bass, bacc, mybir, tile
from concourse import bass_utils
from concourse.ap import AP

F32 = mybir.dt.float32
F32R = mybir.dt.float32r

N_CORES = 8
T_FULL = 16 * 1024          # total tokens
T = T_FULL // N_CORES       # tokens per core = 2048
D = 768                     # in features (contraction)
O = 2304                    # out features
P = 2304                    # total perm (mask length)
PAD = 128                   # ext left wrap pad (covers the intra-tile skew)
EXTW = P + PAD              # 2432
KTOP = 231                  # top-k target
NUM_ITER = 50
INV_LR = 100.0              # 1 / 0.01
K_OVER_N = np.float64(KTOP) / np.float64(P)  # added as fp32 imm by bass

NT = T // 128               # 16 token tiles per core
NJ = D // 128               # 6 contraction tiles
NP = P // 128               # 18 mask/V row tiles
# o-chunks for the main matmul (one PSUM bank each, >=256 for full-rate f32r)
O_CHUNKS = [(0, 512), (512, 1024), (1024, 1536), (1536, 2048), (2048, 2304)]


def build_program():
    nc = bacc.Bacc("TRN2", target_bir_lowering=False, debug=False,
                   num_devices=N_CORES)

    # x/V/ident carry f32 bits; typing them f32r makes the whole transpose +
    # matmul datapath f32r-native (single-pass on the PE).
    x_d = nc.dram_tensor("x", [T, D], F32R, kind="ExternalInput")
    v_d = nc.dram_tensor("v", [P, D], F32R, kind="ExternalInput")
    alpha_d = nc.dram_tensor("alpha", [P], F32, kind="ExternalInput")
    ident_d = nc.dram_tensor("ident", [128, 128], F32R, kind="ExternalInput")
    out_d = nc.dram_tensor("out", [T, O], F32, kind="ExternalOutput")

    ext_d = nc.dram_tensor("ext", [D, EXTW], F32R, kind="Internal")
    mtmp_d = nc.dram_tensor("m_tmp", [P], F32, kind="Internal")
    mrep_d = nc.dram_tensor("m_rep", [130 * P], F32, kind="Internal")

    x_r = x_d.ap().rearrange("(n p) j -> n p j", p=128)      # [16, 128, 768]
    v_r = v_d.ap().rearrange("(n p) j -> n p j", p=128)      # [18, 128, 768]
    out_r = out_d.ap().rearrange("(n p) o -> n p o", p=128)  # [16, 128, 2304]
    alpha_nat = alpha_d.ap().rearrange("(f p) -> f p", p=128)  # [18, 128] rows

    with tile.TileContext(nc) as tc:
        with (
            tc.tile_pool(name="const", bufs=1) as constp,
            tc.tile_pool(name="small", bufs=1) as small,
            tc.tile_pool(name="xstage", bufs=4) as xstage,
            tc.tile_pool(name="vstage", bufs=3) as vstage,
            tc.tile_pool(name="xt", bufs=NJ) as xtp,
            tc.tile_pool(name="vtp", bufs=NJ) as vtp,
            tc.tile_pool(name="wtp", bufs=NJ) as wtp,
            tc.tile_pool(name="orow", bufs=2) as orow,
            tc.tile_pool(name="ps8", bufs=7, space="PSUM") as ps8,
            tc.tile_pool(name="dk", bufs=1, space="PSUM") as dkp,
        ):
            # ---- constants ----
            ident = constp.tile([128, 128], F32R)
            nc.sync.dma_start(ident[:], ident_d.ap())
            ones_inv_n = constp.tile([128, 128], F32)
            nc.vector.memset(ones_inv_n[:], 1.0 / float(P))
            zeros_t = constp.tile([128, NP], F32)
            nc.vector.memset(zeros_t[:], 0.0)

            # ---- Dykstra (critical path; gpsimd DMA ring keeps it isolated)
            alpha_nat_t = small.tile([18, 128], F32, tag="alnat")
            nc.gpsimd.dma_start(alpha_nat_t[:], alpha_nat)
            al_ps = dkp.tile([128, 18], F32, tag="dk")
            nc.tensor.transpose(al_ps[:], alpha_nat_t[:],
                                ident[0:18, 0:18].bitcast(F32))
            w = small.tile([128, NP], F32, tag="w")
            c = small.tile([128, NP], F32, tag="c")
            red = small.tile([128, 1], F32, tag="red")
            m_t = small.tile([128, NP], F32, tag="m")

            # w = 100*alpha ; w += (k - sum(w))/n
            nc.vector.tensor_scalar_mul(w[:], al_ps[:], INV_LR)
            nc.vector.reduce_sum(red[:], w[:], axis=mybir.AxisListType.X)
            s_ps = dkp.tile([128, 1], F32, tag="dk")
            nc.tensor.matmul(s_ps[:], ones_inv_n[:], red[:], start=True, stop=True)
            nc.vector.tensor_scalar(w[:], w[:], s_ps[:], K_OVER_N,
                                    op0=mybir.AluOpType.subtract,
                                    op1=mybir.AluOpType.add)
            for _ in range(NUM_ITER - 1):
                # c = clip(w, 0, 1); red = sum(c)   (single fused DVE op)
                nc.vector.scalar_tensor_tensor(c[:], w[:], 1.0, zeros_t[:],
                                               op0=mybir.AluOpType.min,
                                               op1=mybir.AluOpType.max,
                                               accum_out=red[:])
                s_ps = dkp.tile([128, 1], F32, tag="dk")
                nc.tensor.matmul(s_ps[:], ones_inv_n[:], red[:],
                                 start=True, stop=True)
                nc.vector.tensor_scalar(w[:], w[:], s_ps[:], K_OVER_N,
                                        op0=mybir.AluOpType.subtract,
                                        op1=mybir.AluOpType.add)
            nc.vector.tensor_scalar(m_t[:], w[:], 1.0, 0.0,
                                    op0=mybir.AluOpType.min,
                                    op1=mybir.AluOpType.max)

            # ---- m -> m_ext DRAM (wrap-extended, via PE transpose) ----
            mt_ps = dkp.tile([18, 128], F32, tag="dk")
            nc.tensor.transpose(mt_ps[:], m_t[:], ident[:].bitcast(F32))
            mt_sb = small.tile([18, 128], F32, tag="mtsb")
            nc.vector.tensor_copy(mt_sb[:], mt_ps[:])
            # m_tmp = m_vec (9KB contiguous), then one DRAM->DRAM DMA tiles
            # it 130x into m_rep (skew reads use positive partition stride
            # 2303 == -1 mod 2304)
            mw0 = nc.gpsimd.dma_start(
                mtmp_d.ap().rearrange("(f p) -> f p", p=128), mt_sb[:])
            mw1 = nc.gpsimd.dma_start(
                AP(mrep_d, 0, [[P, 130], [1, P]]),
                AP(mtmp_d, 0, [[0, 130], [1, P]]))
            tile.add_dep_helper(mw1.ins, mw0.ins, reason="m_tmp RAW")

            # ---- V load + transpose:  vt[b][j_local, p] = V[p, j0+j_local] ----
            vt = [vtp.tile([128, P], F32R, tag="vtp", name=f"vt{b}")
                  for b in range(NJ)]
            wt = [wtp.tile([128, P], F32R, tag="wtp", name=f"wt{b}")
                  for b in range(NJ)]
            cp_flip = 0
            for i in range(NP):
                v_t = vstage.tile([128, D], F32R, tag="vstage")
                nc.sync.dma_start(v_t[:], v_r[i])
                for b in range(NJ):
                    ps = ps8.tile([128, 128], F32R, tag="ps8")
                    nc.tensor.transpose(ps[:], v_t[:, 128 * b:128 * (b + 1)],
                                        ident[:])
                    dst = vt[b][:, 128 * i:128 * (i + 1)]
                    if cp_flip % 2 == 0:
                        nc.scalar.copy(dst, ps[:])
                    else:
                        nc.vector.tensor_copy(dst, ps[:])
                    cp_flip += 1

            # ---- x load + transpose:  xt[b][j_local, t] = x[t, j0+j_local] ----
            xt = [xtp.tile([128, T], F32R, tag="xt", name=f"xt{b}")
                  for b in range(NJ)]

            def x_tile_transpose(tt, flip):
                x_t = xstage.tile([128, D], F32R, tag="xstage", name=f"xs{tt}")
                nc.scalar.dma_start(x_t[:], x_r[tt])
                for b in range(NJ):
                    ps = ps8.tile([128, 128], F32R, tag="ps8", name=f"xps{tt}_{b}")
                    nc.tensor.transpose(ps[:], x_t[:, 128 * b:128 * (b + 1)],
                                        ident[:])
                    dst = xt[b][:, 128 * tt:128 * (tt + 1)]
                    if flip % 2 == 0:
                        nc.scalar.copy(dst, ps[:])
                    else:
                        nc.vector.tensor_copy(dst, ps[:])
                    flip += 1
                return flip

            for tt in range(NT):
                cp_flip = x_tile_transpose(tt, cp_flip)

            # ---- raw rolled weights: ext roundtrip (independent of m) ----
            ext_writes = []
            for b in range(NJ):
                j0 = 128 * b
                wmain = nc.sync.dma_start(ext_d.ap()[j0:j0 + 128, PAD:EXTW],
                                          vt[b][:])
                wwrap = nc.sync.dma_start(ext_d.ap()[j0:j0 + 128, 0:PAD],
                                          vt[b][:, P - PAD:P])
                ext_writes.append((wmain, wwrap))
            for b in range(NJ):
                j0 = 128 * b
                # piece A: wt[b][dj, o] for o in [j0, 2304):
                #   ext[j0+dj, PAD + (o - j0) - dj]
                skA = AP(ext_d, j0 * EXTW + PAD, [[EXTW - 1, 128], [1, P - j0]])
                rdA = nc.sync.dma_start(wt[b][:, j0:P], skA)
                tile.add_dep_helper(rdA.ins, ext_writes[b][0].ins, reason="extA")
                tile.add_dep_helper(rdA.ins, ext_writes[b][1].ins, reason="extAw")
                if j0 > 0:
                    # piece B: o in [0, j0): ext[j0+dj, PAD + (o + 2304 - j0) - dj]
                    skB = AP(ext_d, j0 * EXTW + PAD + (P - j0),
                             [[EXTW - 1, 128], [1, j0]])
                    rdB = nc.sync.dma_start(wt[b][:, 0:j0], skB)
                    tile.add_dep_helper(rdB.ins, ext_writes[b][0].ins, reason="extB")
                    tile.add_dep_helper(rdB.ins, ext_writes[b][1].ins, reason="extBw")

            # ---- skewed m broadcast + scale (after Dykstra) ----
            # m_skew[b][dj, o] = m_rep[dj*2303 + o + 2304 - j0]
            #                  = m_vec[(o - dj - j0) mod 2304]
            for b in range(NJ):
                j0 = 128 * b
                msk = vtp.tile([128, P], F32, tag="vtp", name=f"msk{b}")
                mr = nc.gpsimd.dma_start(
                    msk[:], AP(mrep_d, P - j0, [[P - 1, 128], [1, P]]))
                tile.add_dep_helper(mr.ins, mw1.ins, reason="m_rep RAW")
                nc.vector.tensor_tensor(wt[b][:], wt[b][:], msk[:],
                                        op=mybir.AluOpType.mult)

            # ---- main matmul: out[t, o] = sum_j x[t, j] * W.T[j, o] ----
            for tt in range(NT):
                row = orow.tile([128, O], F32, tag="orow")
                for ci, (o0, o1) in enumerate(O_CHUNKS):
                    ps = ps8.tile([128, 512], F32, tag="ps8")
                    cw = o1 - o0
                    for b in range(NJ):
                        nc.tensor.matmul(
                            ps[:, 0:cw],
                            xt[b][:, 128 * tt:128 * (tt + 1)],
                            wt[b][:, o0:o1],
                            start=(b == 0), stop=(b == NJ - 1),
                        )
                    if ci % 2 == 0:
                        nc.vector.tensor_copy(row[:, o0:o1], ps[:, 0:cw])
                    else:
                        nc.scalar.copy(row[:, o0:o1], ps[:, 0:cw])
                nc.scalar.dma_start(out_r[tt], row[:])

    nc.compile()
    return nc


_CACHE = {}


def _get_program():
    if "nc" not in _CACHE:
        _CACHE["nc"] = build_program()
    return _CACHE["nc"]


def kernel(x, V, alpha):
    nc = _get_program()
    xf = np.ascontiguousarray(x.reshape(T_FULL, D).astype(np.float32, copy=False))
    v = np.ascontiguousarray(V.astype(np.float32, copy=False))
    a = np.ascontiguousarray(alpha.astype(np.float32, copy=False))
    ident = np.eye(128, dtype=np.float32)
    in_maps = [
        {"x": xf[T * c:T * (c + 1)], "v": v, "alpha": a, "ident": ident}
        for c in range(N_CORES)
    ]
    res = bass_utils.run_bass_kernel_spmd(nc, in_maps, core_ids=list(range(N_CORES)))
    out = np.concatenate([res.results[c]["out"] for c in range(N_CORES)], axis=0)
    return out.reshape(16, 1024, O)



# revision 10
# speedup vs baseline: 1.5240x; 1.5240x over previous
"""Trainium2 Bass kernel: soft-top-k-masked pseudo-diagonal fully connected layer.

Computes, for x [16, 1024, 768], V [2304, 768], alpha [2304]:
    m  = dykstra_capped_simplex(alpha / 0.01, k=231, 50 iters)        # [2304]
    W[o, j] = m[(o - j) % 2304] * V[(o - j) % 2304, j]                # [2304, 768]
    out = x @ W.T                                                     # [16, 1024, 2304]

Key structure (v2):
  * Host-side layout prep (pure data movement, no arithmetic): x is uploaded
    pre-transposed per-core as xT [768, 2048] fp16; V is uploaded as the
    pre-rolled W_raw [768, 2304] fp16 with W_raw[j, o] = V.T[j, (o-j) % 2304].
    This removes all on-device PE transposes and the DRAM ext roundtrip of the
    previous version.
  * Dykstra reduces to a scalar-bounds recursion: with z = 100*alpha fixed,
        c_i  = clip(z, lo_i, hi_i)          (lo_0 = -inf, hi_0 = +inf)
        A_i  = sum(c_i)
        lo_{i+1} = (A_i - k)/n,   hi_{i+1} = lo_{i+1} + 1
        m = clip(z, lo_50, hi_50) - lo_50
    which is algebraically identical to the reference w-recursion (w_i = z + D_i
    with D_i = -lo_i).  Per iteration: ONE fused DVE clip+accumulate
    (tensor_scalar with two per-partition scalar operands reading the bounds),
    ONE tiny PE matmul against an exact all-ones f32r stationary for the
    cross-partition sum, and ONE tiny DVE op computing both bounds with exact
    f32 constants.  The exact-1.0 stationary matters: the iteration map has
    contraction factor ~0.99, so any systematic per-iteration bias in A/n is
    amplified ~40x into m.
  * The mask is applied to the rolled weights via a skewed broadcast of m
    (m_rep DRAM replicate trick, partition stride 2303 == -1 mod 2304), chunked
    by output columns so the main matmul's first o-chunk starts as soon as m
    and its first msk slices land.
  * Main matmul: o-chunk-major sweeps (5 chunks of <=512), token tiles inner,
    contraction over 6 j-bands accumulated in PSUM.  fp16 operands: 1
    cycle/row streaming, weight loads hidden under the previous matmul.
    Output downloaded fp16 and upcast on host.

Sharding: data-parallel over the 16384 tokens -> 2048 tokens per core on 8
cores; V/alpha replicated (per the sharding hint).
"""

import numpy as np

from concourse import bass, bacc, mybir, tile
from concourse import bass_utils
from concourse.ap import AP

F32 = mybir.dt.float32
F32R = mybir.dt.float32r
F16 = mybir.dt.float16

N_CORES = 8
T_FULL = 16 * 1024          # total tokens
T = T_FULL // N_CORES       # tokens per core = 2048
D = 768                     # in features (contraction)
O = 2304                    # out features
P = 2304                    # total perm (mask length)
KTOP = 231                  # top-k target
NUM_ITER = 50
INV_LR = 100.0              # 1 / 0.01
INV_N = 1.0 / float(P)
KK_LO = float(np.float32(KTOP) / np.float32(P))          # k/n
KK_HI = float((np.float32(KTOP) - np.float32(P)) / np.float32(P))  # (k-n)/n

NT = T // 128               # 16 token tiles per core
NJ = D // 128               # 6 contraction bands
NF = P // 128               # 18 = columns of the [128, 18] dykstra layout
# o-chunks for the main matmul (one PSUM bank each)
O_CHUNKS = [(0, 512), (512, 1024), (1024, 1536), (1536, 2048), (2048, 2304)]
BIG = 1.0e30


def build_program():
    nc = bacc.Bacc("TRN2", target_bir_lowering=False, debug=False,
                   num_devices=N_CORES)

    xT_d = nc.dram_tensor("xT", [D, T], F16, kind="ExternalInput")
    wraw_d = nc.dram_tensor("wraw", [D, O], F16, kind="ExternalInput")
    alphaT_d = nc.dram_tensor("alphaT", [128, NF], F32, kind="ExternalInput")
    ident_d = nc.dram_tensor("ident", [128, 128], F16, kind="ExternalInput")
    ones_d = nc.dram_tensor("ones", [128, 128], F32R, kind="ExternalInput")
    out_d = nc.dram_tensor("out", [T, O], F16, kind="ExternalOutput")

    mtmp_d = nc.dram_tensor("m_tmp", [P], F16, kind="Internal")
    mrep_d = nc.dram_tensor("m_rep", [130 * P], F16, kind="Internal")

    out_r = out_d.ap().rearrange("(n p) o -> n p o", p=128)  # [16, 128, 2304]

    with tile.TileContext(nc) as tc:
        with (
            tc.tile_pool(name="const", bufs=1) as constp,
            tc.tile_pool(name="small", bufs=1) as small,
            tc.tile_pool(name="xt", bufs=NJ) as xtp,
            tc.tile_pool(name="wt", bufs=NJ) as wtp,
            tc.tile_pool(name="msk", bufs=NJ) as mskp,
            tc.tile_pool(name="ost", bufs=6) as ostp,
            tc.tile_pool(name="mmps", bufs=4, space="PSUM") as mmps,
            tc.tile_pool(name="dk", bufs=2, space="PSUM") as dkp,
        ):
            # ---- constants ----
            ident = constp.tile([128, 128], F16)
            nc.gpsimd.dma_start(ident[:], ident_d.ap())
            ones_r = constp.tile([128, 128], F32R)
            nc.gpsimd.dma_start(ones_r[:], ones_d.ap())
            kk2n = constp.tile([128, 2], F32)
            nc.vector.memset(kk2n[:, 0:1], KK_LO)
            nc.vector.memset(kk2n[:, 1:2], KK_HI)

            # ---- input loads (off the dykstra critical path) ----
            al_t = small.tile([128, NF], F32, tag="al")
            nc.gpsimd.dma_start(al_t[:], alphaT_d.ap())
            xt = [xtp.tile([128, T], F16, tag="xt", name=f"xt{b}")
                  for b in range(NJ)]
            wt = [wtp.tile([128, O], F16, tag="wt", name=f"wt{b}")
                  for b in range(NJ)]
            for b in range(NJ):
                nc.scalar.dma_start(xt[b][:], xT_d.ap()[128 * b:128 * (b + 1), :])
            for b in range(NJ):
                nc.sync.dma_start(wt[b][:], wraw_d.ap()[128 * b:128 * (b + 1), :])

            # ---- Dykstra scalar-bounds recursion ----
            z = small.tile([128, NF], F32, tag="z")
            c = small.tile([128, NF], F32R, tag="c")
            red = small.tile([128, 1], F32R, tag="red")
            bounds = small.tile([128, 2], F32, tag="bounds")  # [lo, hi]
            m16 = small.tile([128, NF], F16, tag="m16")

            nc.vector.tensor_scalar_mul(z[:], al_t[:], INV_LR)
            nc.vector.memset(bounds[:, 0:1], -BIG)
            nc.vector.memset(bounds[:, 1:2], BIG)
            lo_bcast = bounds[:, 0:1].broadcast_to([128, NF])
            for i in range(NUM_ITER):
                # c = (z min hi) max lo ; red = per-partition row sums
                nc.vector.scalar_tensor_tensor(c[:], z[:], bounds[:, 1:2],
                                               lo_bcast,
                                               op0=mybir.AluOpType.min,
                                               op1=mybir.AluOpType.max,
                                               accum_out=red[:])
                a_ps = dkp.tile([128, 2], F32, tag="dk")
                nc.tensor.matmul(a_ps[:], ones_r[:],
                                 red[:].broadcast_to([128, 2]),
                                 start=True, stop=True)
                # bounds = (A * 1/n) - [k/n, (k-n)/n]
                nc.vector.scalar_tensor_tensor(bounds[:], a_ps[:],
                                               INV_N, kk2n[:],
                                               op0=mybir.AluOpType.mult,
                                               op1=mybir.AluOpType.subtract)
            # m = clip(z, lo, hi) - lo   (fresh clip with final bounds)
            nc.vector.scalar_tensor_tensor(c[:], z[:], bounds[:, 1:2],
                                           lo_bcast,
                                           op0=mybir.AluOpType.min,
                                           op1=mybir.AluOpType.max)
            nc.vector.tensor_scalar(m16[:], c[:], bounds[:, 0:1], None,
                                    op0=mybir.AluOpType.subtract)

            # ---- m -> DRAM natural order -> 130x replicate for skewed reads
            mt_ps = dkp.tile([NF, 128], F16, tag="dk")
            nc.tensor.transpose(mt_ps[:], m16[:], ident[:])
            mt_sb = small.tile([NF, 128], F16, tag="mtsb")
            nc.scalar.copy(mt_sb[:], mt_ps[:])
            mw0 = nc.gpsimd.dma_start(
                mtmp_d.ap().rearrange("(f p) -> f p", p=128), mt_sb[:])
            mw1 = nc.gpsimd.dma_start(
                AP(mrep_d, 0, [[P, 130], [1, P]]),
                AP(mtmp_d, 0, [[0, 130], [1, P]]))
            tile.add_dep_helper(mw1.ins, mw0.ins, reason="m_tmp RAW")

            # ---- skewed m broadcast: msk[b][dj, o] = m[(o - dj - j0) % P]
            msk = [mskp.tile([128, O], F16, tag="msk", name=f"msk{b}")
                   for b in range(NJ)]
            mq = [nc.gpsimd, nc.gpsimd, nc.gpsimd, nc.sync, nc.sync, nc.sync]
            # first o-chunk slices for all bands first, then the rest
            for b in range(NJ):
                j0 = 128 * b
                r = mq[b].dma_start(
                    msk[b][:, 0:512],
                    AP(mrep_d, P - j0, [[P - 1, 128], [1, 512]]))
                tile.add_dep_helper(r.ins, mw1.ins, reason="m_rep RAW")
            for b in range(NJ):
                j0 = 128 * b
                r = mq[b].dma_start(
                    msk[b][:, 512:O],
                    AP(mrep_d, P - j0 + 512, [[P - 1, 128], [1, O - 512]]))
                tile.add_dep_helper(r.ins, mw1.ins, reason="m_rep RAW")

            # ---- apply mask: wt[b] *= msk[b], chunk-major so chunk 0 is ready
            # first; alternate vector/gpsimd
            for ci, (o0, o1) in enumerate(O_CHUNKS):
                for b in range(NJ):
                    eng = nc.vector if (ci * NJ + b) % 2 == 0 else nc.gpsimd
                    eng.tensor_tensor(wt[b][:, o0:o1], wt[b][:, o0:o1],
                                      msk[b][:, o0:o1],
                                      op=mybir.AluOpType.mult)

            # ---- main matmul: o-chunk sweeps, token tiles inner ----
            flip = 0
            for ci, (o0, o1) in enumerate(O_CHUNKS):
                cw = o1 - o0
                for tt in range(NT):
                    ps = mmps.tile([128, 512], F32, tag="mm")
                    for b in range(NJ):
                        nc.tensor.matmul(
                            ps[:, 0:cw],
                            xt[b][:, 128 * tt:128 * (tt + 1)],
                            wt[b][:, o0:o1],
                            start=(b == 0), stop=(b == NJ - 1),
                        )
                    ost = ostp.tile([128, 512], F16, tag="ost")
                    if flip % 2 == 0:
                        nc.scalar.copy(ost[:, 0:cw], ps[:, 0:cw])
                        nc.scalar.dma_start(out_r[tt][:, o0:o1], ost[:, 0:cw])
                    else:
                        nc.vector.tensor_copy(ost[:, 0:cw], ps[:, 0:cw])
                        nc.sync.dma_start(out_r[tt][:, o0:o1], ost[:, 0:cw])
                    flip += 1

    nc.compile()
    return nc


_CACHE = {}


def _get_program():
    if "nc" not in _CACHE:
        _CACHE["nc"] = build_program()
    return _CACHE["nc"]


def _host_inputs(x, V, alpha):
    """Pure layout prep (transpose/cast/roll); no arithmetic on values."""
    xf = np.ascontiguousarray(x.reshape(T_FULL, D))
    VT16 = np.ascontiguousarray(V.T.astype(np.float16))          # [768, 2304]
    idx = (np.arange(O)[None, :] - np.arange(D)[:, None]) % P    # [768, 2304]
    wraw = np.ascontiguousarray(np.take_along_axis(VT16, idx, axis=1))
    alphaT = np.ascontiguousarray(
        alpha.astype(np.float32).reshape(NF, 128).T)             # [128, 18]
    ident = np.eye(128, dtype=np.float16)
    ones = np.ones((128, 128), dtype=np.float32)
    maps = []
    for cid in range(N_CORES):
        xT = np.ascontiguousarray(
            xf[T * cid:T * (cid + 1)].T.astype(np.float16))      # [768, 2048]
        maps.append({"xT": xT, "wraw": wraw, "alphaT": alphaT,
                     "ident": ident, "ones": ones})
    return maps


def kernel(x, V, alpha):
    nc = _get_program()
    in_maps = _host_inputs(x, V, alpha)
    res = bass_utils.run_bass_kernel_spmd(nc, in_maps,
                                          core_ids=list(range(N_CORES)))
    out = np.concatenate(
        [res.results[c]["out"].astype(np.float32) for c in range(N_CORES)],
        axis=0)
    return out.reshape(16, 1024, O)
